# revision 22
# baseline (speedup 1.0000x reference)
"""Trainium2 Bass kernel for the ExportableStudentSNN1d problem (fp8 conv2,
LIF carry recursion folded into the PE via a DoubleRow identity pair).

Data-parallel over batch: 64 samples -> 8 cores x 8 samples. Each core runs
an identical NEFF on its batch shard; host concatenates the [8, 4] outputs.

Math notes (TAU1 = 1.0 makes layer-1 LIF memoryless):
  s1_t = (conv1(x_t)*G + b1*G >= TH1)        <=> conv1(x_t) >= TH1/G - b1
  s1 is stored +/-1-encoded: sgn = Sign(conv1 - th1) (ACT engine), and
  since s1 = (sgn+1)/2 is linear, conv2 runs on sgn with HALVED weights
  (exact power-of-2 scale in fp8) and 0.5*sum(W2q) folded into b2p.
  layer2 runs in a 2^4-scaled domain (all linear + thresholds scale):
     psum2 = conv2(sgn; W2*(10/9)*G*2^4/2 fp8) + alpha*c8   [all on PE]
     y     = psum2 + b2p
     spike count via ACT Sign(y - TH2'), accum_out
     c8    = fp8(y * (y < TH2'))   [next step's carry, fused DVE stt]
  where alpha = fp8(-1/9) = -7/64 rides slot 1 of the tap-8 DoubleRow pair
  as alpha*I, with the carry c8 as that slot's moving operand. The decay
  error (-7/64 vs -1/9) + fp8 carry rounding measure ~1.5e-3 end to end.
  fp8 weight-quantization DC error is cancelled by folding
  -sum_{ci,k} dW[co,ci,k] * E[s1[ci]] into b2 (E[s1[ci]] analytic: conv1
  output is Gaussian with sigma = ||W1[ci]||_F).
  out[b,c] = (sum_{t,l} sp)/(T*L) @ Wfc.T + bfc

conv2 is 5 fp8 DoubleRow matmuls per 512-chunk: taps (0,1),(2,3),(4,5),
(6,7) + (tap8, alpha*I). The s1 tile is [128, 4, S1P] fp8 with slabs
[sgn, c8(h0), sgn<<1, c8(h1)]; tap pairs read slabs {0,2}, the carry pair
{0,1} (h0) / {0,3} (h1). slab2 = slab0 shifted one column via a SBUF->SBUF
DMA on the qAct HWDGE queue (so it never queues behind x staging on qSP).
conv1 stays bf16 with a DMA-materialized im2col.
"""

import math

import numpy as np
import ml_dtypes

import concourse.bacc as bacc
import concourse.tile as tile
import concourse.mybir as mybir
from concourse.bass_utils import run_bass_kernel_spmd

F32 = mybir.dt.float32
BF16 = mybir.dt.bfloat16
FP8 = mybir.dt.float8e4
E4 = ml_dtypes.float8_e4m3

N_CORES = 8
B, C_IN, L, T = 64, 12, 2048, 20
C1, C2, K, PAD = 128, 256, 9, 4
GAIN, TAU2, TH1, TH2 = 3.0, 0.9, 0.02, 0.02
NCLS = 4
B_SH = B // N_CORES            # 8 samples per core
LH = 1024                      # L processed in halves
HALO = 8                       # x halo per side (conv1 then conv2 shifts)
S1W = LH + 2 * PAD             # 1032 s1 columns needed per L-half
S1P = 1040                     # s1 slab width (mult of 16 for DR pair step)
A2S = (10.0 / 9.0) * GAIN      # 10/3: multiplier on conv2 psum
SC2 = 16.0                     # layer-2 scale; keeps y, c8 in fp8 range
TH2S = TH2 * SC2
ALPHA = -7.0 / 64.0            # fp8(-1/9): carry decay inside the PE

_CACHE = {}


def _build():
    nc = bacc.Bacc("TRN2", target_bir_lowering=False, debug=False)

    # x arrives HOST-side im2col'd: row (12k+ci) of [b, lh] holds
    # x[b, ci, t, lh*1024 + c + k - 8] (zero-padded at L edges), so staging
    # a segment is ONE contiguous 4.5MB DMA (108 runs of 41KB) instead of
    # ~4400 2KB descriptors that saturate the rings for the whole segment.
    x_d = nc.dram_tensor(
        "x", [B_SH, 2, K * C_IN, T * S1W], BF16, kind="ExternalInput")
    w1t_d = nc.dram_tensor("w1t", [K * C_IN, C1], BF16, kind="ExternalInput")
    w2dr_d = nc.dram_tensor("w2dr", [C1, 2, 8 * C1], FP8, kind="ExternalInput")
    w285_d = nc.dram_tensor("w285", [C1, 2, C2], FP8, kind="ExternalInput")
    nth1_d = nc.dram_tensor("nth1", [C1, 1], F32, kind="ExternalInput")
    b2p_d = nc.dram_tensor("b2p", [C1, 2], F32, kind="ExternalInput")
    wfc_d = nc.dram_tensor("wfc", [C1, 2 * NCLS], F32, kind="ExternalInput")
    bfc_d = nc.dram_tensor("bfc", [NCLS, 1], F32, kind="ExternalInput")
    out_d = nc.dram_tensor("out", [B_SH, NCLS], F32, kind="ExternalOutput")

    with tile.TileContext(nc) as tc:
        with (
            tc.tile_pool(name="const", bufs=1) as cpool,
            tc.tile_pool(name="xstage", bufs=2) as xpool,
            tc.tile_pool(name="s1", bufs=2) as s1pool,
            tc.tile_pool(name="lif", bufs=3) as lifpool,
            tc.tile_pool(name="psum1", bufs=1, space="PSUM") as pp1,
            tc.tile_pool(name="psum2", bufs=2, space="PSUM") as pp2,
            tc.tile_pool(name="psfc", bufs=1, space="PSUM") as ppfc,
        ):
            # ---- constants / weights (resident) ----
            # w1t rows (12k+ci) hold W1[:, ci, k] (im2col layout)
            w1t = cpool.tile([K * C_IN, C1], BF16)
            nc.sync.dma_start(w1t[:], w1t_d.ap())
            # DR pairs: w2dr[ci, i, (j*2+h)*128+co] = W2h[h*128+co, ci, 2j+i]
            w2dr = cpool.tile([C1, 2, 8 * C1], FP8)
            nc.sync.dma_start(w2dr[:], w2dr_d.ap())
            # 5th pair: slot0 = tap 8 of W2h, slot1 = alpha*I (carry decay)
            w285 = cpool.tile([C1, 2, C2], FP8)
            nc.sync.dma_start(w285[:], w285_d.ap())
            nth1 = cpool.tile([C1, 1], F32)
            nc.sync.dma_start(nth1[:], nth1_d.ap())
            b2p = cpool.tile([C1, 2], F32)
            nc.sync.dma_start(b2p[:], b2p_d.ap())
            nth2 = cpool.tile([C1, 1], F32)
            nc.gpsimd.memset(nth2[:], -TH2S)
            wfc = cpool.tile([C1, 2 * NCLS], F32)
            nc.sync.dma_start(wfc[:], wfc_d.ap())
            bfc = cpool.tile([NCLS, 1], F32)
            nc.sync.dma_start(bfc[:], bfc_d.ap())
            # spike counts, one column per (h, b, lh, t)
            acc = cpool.tile([C1, 2 * B_SH * 2 * T], F32)

            segs = [(b, lh) for b in range(B_SH) for lh in range(2)]

            def stage_segment(idx):
                # host-side im2col: one contiguous DMA per segment
                b, lh = segs[idx]
                xs = xpool.tile([K * C_IN, T * S1W], BF16)
                src = x_d.ap()[b, lh]
                if idx == 0:
                    # cold start: split so the first timesteps' columns
                    # land first
                    nc.sync.dma_start(xs[:, 0 : 2 * S1W], src[:, 0 : 2 * S1W])
                    nc.sync.dma_start(xs[:, 2 * S1W :], src[:, 2 * S1W :])
                else:
                    nc.sync.dma_start(xs[:], src)
                return xs

            def conv1_block(xs, t, first):
                # conv1: one K=108 matmul per chunk. s1 slab0 = Sign(p1-th1)
                # on ACT (+/-1 encoding); slab2 = slab0 shifted one column
                # via SBUF->SBUF DMA on the qAct queue. Cols >= 1032 are
                # junk (stale psum -> +/-1) but never read by conv2.
                # Carry slabs 1/3 are zeroed at segment starts (v2 init).
                p1 = pp1.tile([C1, 1536], F32)
                for c0, cn in ((0, 512), (512, 512), (1024, S1W - 1024)):
                    nc.tensor.matmul(
                        p1[:, c0 : c0 + cn],
                        w1t[:],
                        xs[:, t * S1W + c0 : t * S1W + c0 + cn],
                        start=True,
                        stop=True,
                    )
                s1 = s1pool.tile([C1, 4, S1P], FP8)
                if first:
                    nc.gpsimd.memset(s1[:, 1], 0.0)
                    nc.gpsimd.memset(s1[:, 3], 0.0)
                nc.scalar.activation(
                    s1[:, 0], p1[:, 0:S1P],
                    mybir.ActivationFunctionType.Sign,
                    bias=nth1[:],
                )
                if first:
                    # segment-boundary tiles are produced late in the phase;
                    # the ~10us SBUF->SBUF DMA latency would stall the PE,
                    # so compute the shifted slab directly on ACT instead
                    nc.scalar.activation(
                        s1[:, 2, 0 : S1P - 1], p1[:, 1:S1P],
                        mybir.ActivationFunctionType.Sign,
                        bias=nth1[:],
                    )
                else:
                    nc.scalar.dma_start(s1[:, 2, 0 : S1P - 1], s1[:, 0, 1:S1P])
                return s1

            def conv2_block(s1, h):
                # 5 DR matmuls per 512-chunk: 4 tap pairs on slabs {0,2} +
                # the (tap8, alpha*I) pair on slabs {0, 1+2h} -- slot 1's
                # moving operand is the previous step's carry c8(h).
                p2 = pp2.tile([C1, LH], F32)
                for c0 in (0, 512):
                    for j in range(4):
                        nc.tensor.matmul(
                            p2[:, c0 : c0 + 512],
                            w2dr[:, 0:2, (j * 2 + h) * C1 : (j * 2 + h + 1) * C1],
                            s1[:, 0:3:2, c0 + 2 * j : c0 + 2 * j + 512],
                            start=(j == 0),
                            stop=False,
                            perf_mode=mybir.MatmulPerfMode.DoubleRow,
                        )
                    nc.tensor.matmul(
                        p2[:, c0 : c0 + 512],
                        w285[:, 0:2, h * C1 : (h + 1) * C1],
                        (s1[:, 0:2:1, c0 + 8 : c0 + 8 + 512] if h == 0
                         else s1[:, 0:4:3, c0 + 8 : c0 + 8 + 512]),
                        start=False,
                        stop=True,
                        perf_mode=mybir.MatmulPerfMode.DoubleRow,
                    )
                return p2

            def lif_step(p2, s1_next, h, col):
                # y = psum2 + b2p (psum already holds the carry term);
                # sign-sum spike count on ACT; next carry c8 = y*(y<TH2')
                # fused on DVE, written into the NEXT s1 tile's carry slab
                # (offset +8 so the tap-8 window alignment matches).
                y = lifpool.tile([C1, LH], F32, tag="y")
                nc.vector.tensor_scalar(
                    y[:], p2[:], b2p[:, h : h + 1], None,
                    op0=mybir.AluOpType.add,
                )
                sg = lifpool.tile([C1, LH], F32, tag="sg")
                nc.scalar.activation(
                    sg[:], y[:], mybir.ActivationFunctionType.Sign,
                    bias=nth2[:],
                    accum_out=acc[:, col + h * (B_SH * 2 * T) :
                                  col + h * (B_SH * 2 * T) + 1],
                )
                if s1_next is not None:
                    nc.vector.scalar_tensor_tensor(
                        s1_next[:, 1 + 2 * h, 8 : 8 + LH], y[:], TH2S, y[:],
                        op0=mybir.AluOpType.is_lt, op1=mybir.AluOpType.mult,
                    )

            # conv1 of segment idx+1's t=0 fills the empty t=19 pipeline
            # slot of segment idx, so segment boundaries don't stall PE
            staged = {0: stage_segment(0)}
            s1_cur = conv1_block(staged[0], 0, True)
            for idx in range(len(segs)):
                b, lh = segs[idx]
                xs = staged.pop(idx)
                if idx + 1 < len(segs):
                    staged[idx + 1] = stage_segment(idx + 1)
                for t in range(T):
                    col = b * (2 * T) + lh * T + t
                    p2_0 = conv2_block(s1_cur, 0)
                    # emit conv1(t+1)+s1(t+1) before the h0 LIF ops: s1 and
                    # the carry-slab target land early so conv2(t+1) and the
                    # c8 writes never wait
                    last_t = t == T - 1
                    if not last_t:
                        s1_next = conv1_block(xs, t + 1, False)
                    elif idx + 1 < len(segs):
                        s1_next = conv1_block(staged[idx + 1], 0, True)
                    else:
                        s1_next = None
                    # carry dies at segment end (v2 resets per sample/half)
                    c8_tgt = None if last_t else s1_next
                    lif_step(p2_0, c8_tgt, 0, col)
                    p2_1 = conv2_block(s1_cur, 1)
                    lif_step(p2_1, c8_tgt, 1, col)
                    s1_cur = s1_next

            # ---- pooling + FC head ----
            pooled = cpool.tile([C1, 2 * B_SH], F32)
            nc.vector.tensor_reduce(
                pooled[:],
                acc[:].rearrange("p (h b c) -> p (h b) c", h=2, b=B_SH),
                axis=mybir.AxisListType.X, op=mybir.AluOpType.add,
            )
            pfc = ppfc.tile([NCLS, B_SH], F32)
            for h in range(2):
                nc.tensor.matmul(
                    pfc[:],
                    wfc[:, h * NCLS : (h + 1) * NCLS],
                    pooled[:, h * B_SH : (h + 1) * B_SH],
                    start=(h == 0),
                    stop=(h == 1),
                )
            # pfc holds Wfc @ sign_sums; counts = (sign_sum + T*L)/2 is folded
            # into scale and the host-adjusted bias
            fin = cpool.tile([NCLS, B_SH], F32)
            nc.scalar.activation(
                fin[:], pfc[:], mybir.ActivationFunctionType.Identity,
                bias=bfc[:], scale=1.0 / float(2 * T * L),
            )
            nc.sync.dma_start(out_d.ap().rearrange("b c -> c b"), fin[:])

    nc.compile()
    return nc


def _prep_consts(W1, b1, W2, b2, Wfc, bfc):
    # w1t im2col layout: row (12k+ci), col co = W1[co, ci, k]
    w1t = np.ascontiguousarray(W1.transpose(2, 1, 0)).reshape(K * C_IN, C1)
    # W2 pre-scaled to the 2^4 domain, fp8-e4m3 quantized, then HALVED
    # (exact in fp8) for the +/-1 s1 encoding
    w2q8 = (W2.astype(np.float64) * (A2S * SC2)).astype(np.float32).astype(E4)
    w2h = (w2q8.astype(np.float32) * 0.5).astype(E4)   # exact halving
    wt = np.ascontiguousarray(w2h.transpose(1, 2, 0))  # [C1, K, C2]
    w2dr = np.zeros((C1, 2, 8 * C1), dtype=E4)
    for j in range(4):
        for i in range(2):
            for h in range(2):
                w2dr[:, i, (j * 2 + h) * C1 : (j * 2 + h + 1) * C1] = (
                    wt[:, 2 * j + i, h * C1 : (h + 1) * C1])
    # 5th pair: slot0 = tap 8 (on sgn), slot1 = alpha*I (on carry c8)
    w285 = np.zeros((C1, 2, C2), dtype=E4)
    w285[:, 0, :] = wt[:, 8, :]
    eye = (np.eye(C1, dtype=np.float32) * ALPHA).astype(E4)
    w285[:, 1, 0:C1] = eye
    w285[:, 1, C1:C2] = eye
    # DC correction: E[s1[ci]] = Phi((b1 - TH1/G)/sigma), sigma = ||W1[ci]||
    sig = np.sqrt((W1.astype(np.float64) ** 2).sum(axis=(1, 2)))
    z = (b1.astype(np.float64) - TH1 / GAIN) / sig
    p_ci = np.array([0.5 * (1.0 + math.erf(v / math.sqrt(2.0))) for v in z])
    dw = w2q8.astype(np.float64) / (A2S * SC2) - W2.astype(np.float64)
    b2c = b2.astype(np.float64) - np.einsum("oik,i->o", dw, p_ci)
    # +/-1 encoding: psum = conv_q(s1) - S/2, S = sum of (quantized,
    # unhalved) weights over (ci, k); fold S/2 into b2p
    S = w2q8.astype(np.float64).sum(axis=(1, 2))
    nth1 = -(TH1 / GAIN - b1).reshape(C1, 1).astype(np.float32)
    b2p_full = (A2S * SC2 * b2c + 0.5 * S).astype(np.float32)
    b2p = b2p_full.reshape(2, C1).T.copy()            # [128, 2] cols = halves
    wfcT = Wfc.T.reshape(2, C1, NCLS)                 # [2, 128, 4]
    wfc_t = wfcT.transpose(1, 0, 2).reshape(C1, 2 * NCLS).copy()
    # counts = (sign_sum + T*L)/2 folded into the FC epilogue:
    # out = (Wfc @ sign_sum)/(2*T*L) + (bfc + 0.5*rowsum(Wfc))
    bfc_c = (bfc + 0.5 * Wfc.sum(axis=1)).reshape(NCLS, 1).astype(np.float32)
    return {
        "w1t": w1t.astype(ml_dtypes.bfloat16),
        "w2dr": w2dr,
        "w285": w285,
        "nth1": nth1,
        "b2p": b2p,
        "wfc": wfc_t.astype(np.float32),
        "bfc": bfc_c,
    }


def kernel(x, W1, b1, W2, b2, Wfc, bfc, _trace=False):
    x = np.asarray(x, dtype=np.float32)
    # [B, Cin, L, T] -> [B, Cin, T, L] bf16 so on-chip reads are unit-stride
    x_t = np.ascontiguousarray(x.transpose(0, 1, 3, 2)).astype(ml_dtypes.bfloat16)
    # host-side im2col: xim[b, lh, 12k+ci, t, c] = x[b, ci, t, lh*LH+c+k-8]
    # (zero outside [0, L)) -- turns on-chip staging into one big DMA
    xim = np.zeros((B, 2, K * C_IN, T, S1W), dtype=ml_dtypes.bfloat16)
    for lh in range(2):
        l0 = lh * LH
        for k in range(K):
            c_lo = max(0, HALO - k - l0)
            c_hi = min(S1W, L - l0 - k + HALO)
            xim[:, lh, C_IN * k : C_IN * (k + 1), :, c_lo:c_hi] = (
                x_t[:, :, :, l0 + c_lo + k - HALO : l0 + c_hi + k - HALO])
    xim = xim.reshape(B, 2, K * C_IN, T * S1W)
    consts = _prep_consts(
        np.asarray(W1, np.float32), np.asarray(b1, np.float32),
        np.asarray(W2, np.float32), np.asarray(b2, np.float32),
        np.asarray(Wfc, np.float32), np.asarray(bfc, np.float32),
    )
    if "nc" not in _CACHE:
        _CACHE["nc"] = _build()
    nc = _CACHE["nc"]

    in_maps = []
    for c in range(N_CORES):
        m = dict(consts)
        m["x"] = np.ascontiguousarray(xim[c * B_SH : (c + 1) * B_SH])
        in_maps.append(m)

    res = run_bass_kernel_spmd(
        nc, in_maps, core_ids=list(range(N_CORES)), trace=_trace
    )
    out = np.concatenate([res.results[c]["out"] for c in range(N_CORES)], axis=0)
    out = out.astype(np.float32)
    if _trace:
        return out, res
    return out
